# revision 1
# baseline (speedup 1.0000x reference)
"""Single-head causal attention (B=8, T=2048, C=1024, H=128) on 8 trn2 cores.

Data-parallel over batch: core b computes attention for batch element b.

v3 changes vs v2:
  - startup DMA: few big posts (Wq | Wkv | one 1MB post per x segment,
    s0 split across both HWDGE rings) -> first real matmul ~5us
  - warmup sized to cover DMA landing (64 x N=128, ends ~5us, warm clock)
  - softmax denominators l OFF the PE: diagonal e-tiles zero-filled, all
    strips full-width, bf16 binary-tree adds on VectorE, one ones-matmul
    per q-range (emitted BEFORE PV so the kernel tail is just PV+copy+DMA)
  - l DMA on sync (HWDGE) instead of gpsimd (SWDGE)

Per-core device algorithm (bf16 matmuls, f32 PSUM accum):
  1. qT/kT/vT segments [128, 512] = W.T @ xT   (8 cc chunks each)
  2. v_nat[kt] [128,128] via XBAR dma transpose of vT segment
  3. per q-range r (512 wide):
       full strips kt<4r:  ST=k.q [128,512]; E=exp(ST/sqrt(C))
       diag strips j=0..3: cols [128j,512) computed; [0,128j) memset 0;
                           triangular mask on cols [128j, 128j+128)
       l[r] = ones.T @ (bf16 tree-sum of E strips)     (tree on DVE)
       outT[r] += v_nat[kt].T @ E[kt]   (PSUM accum over strips)
  4. DMA outT[r] (via DVE copy) and l[r] -> DRAM; host does (outT/l).T
"""

import numpy as np

import concourse.bacc as bacc
import concourse.mybir as mybir
import concourse.tile as tile
from concourse.bass_utils import run_bass_kernel_spmd

B, T, C, H = 8, 2048, 1024, 128
NCORES = 8
QR = 512          # q-range width (one PSUM bank)
NQR = T // QR     # 4 q-ranges
NKT = T // 128    # 16 k-strips
NCC = C // 128    # 8 contraction chunks
SCALE = 1.0 / np.sqrt(C)
NWARM = 64        # warmup matmuls (N=128): ~3.4us cold + ~1.7us warm

F32 = mybir.dt.float32
BF16 = mybir.dt.bfloat16


def _build_program():
    nc = bacc.Bacc("TRN2", target_bir_lowering=False, debug=False,
                   num_devices=NCORES, num_swdge_queues=4)

    # x prepped as [s][128 p][cc][512]; one 1MB post per segment
    x_d = nc.dram_tensor("x", [NQR, 128, NCC, QR], BF16, kind="ExternalInput")
    w_d = nc.dram_tensor("w", [128, 3 * NCC * H], BF16, kind="ExternalInput")
    mask_d = nc.dram_tensor("mask", [128, 128], BF16, kind="ExternalInput")
    ones_d = nc.dram_tensor("ones", [128, 1], BF16, kind="ExternalInput")
    out_d = nc.dram_tensor("out", [H, T], F32, kind="ExternalOutput")
    l_d = nc.dram_tensor("l", [1, T], F32, kind="ExternalOutput")

    with tile.TileContext(nc) as tc:
        with (
            tc.tile_pool(name="consts", bufs=1) as consts,
            tc.tile_pool(name="xt", bufs=NQR) as xt_pool,
            tc.tile_pool(name="qkvT", bufs=1) as qkvT_pool,
            tc.tile_pool(name="vnat", bufs=NQR) as vnat_pool,
            tc.tile_pool(name="e", bufs=32) as e_pool,
            tc.tile_pool(name="ep", bufs=16) as ep_pool,
            tc.tile_pool(name="osmall", bufs=1) as osmall_pool,
            tc.tile_pool(name="mm1k", bufs=2, space="PSUM") as mm1k_pool,
            tc.tile_pool(name="st", bufs=3, space="PSUM") as st_pool,
            tc.tile_pool(name="acc", bufs=1, space="PSUM") as acc_pool,
        ):
            # ---- DMA loads: few big posts, s0 split across both rings ----
            w_sb = consts.tile([128, 3, NCC, H], BF16, tag="w")
            w_ap = w_d.ap().rearrange("p (w cc h) -> p w cc h", w=3, cc=NCC)
            xt = [xt_pool.tile([128, NCC, QR], BF16, tag="xt", name=f"xt{s}")
                  for s in range(NQR)]

            # sync ring: Wq, s0a, s1, s3 / scalar ring: s0b, Wkv, s2
            nc.sync.dma_start(w_sb[:, 0:1], w_ap[:, 0:1])
            nc.scalar.dma_start(xt[0][:, 0:4, :], x_d.ap()[0, :, 0:4, :])
            nc.sync.dma_start(xt[0][:, 4:8, :], x_d.ap()[0, :, 4:8, :])
            nc.scalar.dma_start(w_sb[:, 1:3], w_ap[:, 1:3])
            nc.sync.dma_start(xt[1][:], x_d.ap()[1])
            nc.scalar.dma_start(xt[2][:], x_d.ap()[2])
            nc.sync.dma_start(xt[3][:], x_d.ap()[3])
            mask_sb = consts.tile([128, 128], BF16, tag="mask")
            nc.gpsimd.dma_start(mask_sb[:], mask_d.ap())
            ones_sb = consts.tile([128, 1], BF16, tag="ones")
            nc.gpsimd.dma_start(ones_sb[:], ones_d.ap())

            # ---- PE/ACT warmup while DMAs land -----------------------------
            dummyw = consts.tile([128, 128], BF16, tag="dummyw")
            dummyx = consts.tile([128, 128], BF16, tag="dummyx")
            nc.vector.memset(dummyw[:], 1.0)
            nc.vector.memset(dummyx[:], 0.0)
            warm_ps = mm1k_pool.tile([128, QR], F32, tag="mm1k")
            for _ in range(NWARM):
                nc.tensor.matmul(warm_ps[:, 0:128], dummyw[:], dummyx[:],
                                 start=True, stop=True)
            nc.scalar.activation(
                dummyw[:, 0:1], dummyx[:, 0:1],
                mybir.ActivationFunctionType.Exp)

            # ---- qT/kT/vT segments ----------------------------------------
            qTs = [qkvT_pool.tile([128, QR], BF16, tag=f"qT{s}",
                                  name=f"qT{s}") for s in range(NQR)]
            kTs = [qkvT_pool.tile([128, QR], BF16, tag=f"kT{s}",
                                  name=f"kT{s}") for s in range(NQR)]
            vTs = [qkvT_pool.tile([128, QR], BF16, tag=f"vT{s}",
                                  name=f"vT{s}") for s in range(NQR)]

            def kslice(kt):
                return kTs[kt // 4][:, 128 * (kt % 4):128 * (kt % 4 + 1)]

            def emit_qkv(s):
                for wi, dst in ((0, qTs[s]), (1, kTs[s]), (2, vTs[s])):
                    ps = mm1k_pool.tile([128, QR], F32, tag="mm1k")
                    for cc in range(NCC):
                        nc.tensor.matmul(
                            ps[:],
                            w_sb[:, wi, cc, :],
                            xt[s][:, cc, :],
                            start=(cc == 0), stop=(cc == NCC - 1))
                    nc.vector.tensor_copy(dst[:], ps[:])

            # v natural layout via XBAR dma transpose:
            # vnat[p, j, c] = vT[c, 128j + p]
            vnat = [None] * NQR

            def emit_vtr(seg):
                vt = vnat_pool.tile([128, 4, 128], BF16, tag="vnat",
                                    name=f"vnat{seg}")
                nc.sync.dma_start_transpose(vt[:], vTs[seg][:])
                vnat[seg] = vt

            def vslice(kt):
                return vnat[kt // 4][:, kt % 4, :]

            # ---- attention -------------------------------------------------
            all_e = {}

            def emit_st(r):
                nkt = 4 * r + 4
                es = [None] * nkt
                # diagonal strips first so exp+mask clear early
                for kt in list(range(4 * r, nkt)) + list(range(4 * r)):
                    j = kt - 4 * r
                    off = 128 * j if j >= 0 else 0
                    st = st_pool.tile([128, QR], F32, tag="st")
                    nc.tensor.matmul(
                        st[:, off:QR],
                        kslice(kt),
                        qTs[r][:, off:QR],
                        start=True, stop=True)
                    e = e_pool.tile([128, QR], BF16, tag="e",
                                    name=f"e{r}_{kt}")
                    if j >= 1:
                        # dead region zeroed so l-tree adds are full-width
                        nc.vector.memset(e[:, 0:off], 0.0)
                    nc.scalar.activation(
                        e[:, off:QR], st[:, off:QR],
                        mybir.ActivationFunctionType.Exp,
                        scale=float(SCALE))
                    if j >= 0:
                        nc.vector.tensor_mul(
                            e[:, off:off + 128],
                            e[:, off:off + 128],
                            mask_sb[:])
                    es[kt] = e
                all_e[r] = es

            def emit_lsum(r):
                # bf16 binary tree on DVE, then one ones-matmul on PE
                lvl = list(all_e[r])
                while len(lvl) > 1:
                    nxt = []
                    for i in range(0, len(lvl) - 1, 2):
                        dst = ep_pool.tile([128, QR], BF16, tag="ep")
                        nc.vector.tensor_add(dst[:], lvl[i][:], lvl[i + 1][:])
                        nxt.append(dst)
                    if len(lvl) % 2:
                        nxt.append(lvl[-1])
                    lvl = nxt
                l_ps = acc_pool.tile([1, QR], F32, tag="lacc")
                nc.tensor.matmul(l_ps[:], ones_sb[:], lvl[0][:],
                                 start=True, stop=True)
                ls = osmall_pool.tile([1, QR], F32, tag=f"l{r}",
                                      name=f"l{r}")
                nc.scalar.copy(ls[:], l_ps[:])
                nc.sync.dma_start(l_d.ap()[:, QR * r:QR * (r + 1)], ls[:])

            def emit_pv(r):
                nkt = 4 * r + 4
                es = all_e.pop(r)
                o_ps = acc_pool.tile([128, QR], F32, tag="outT")
                for i, kt in enumerate(range(nkt)):
                    j = kt - 4 * r
                    off = 128 * j if j >= 0 else 0
                    nc.tensor.matmul(
                        o_ps[:, off:QR],
                        vslice(kt),
                        es[kt][:, off:QR],
                        start=(i == 0), stop=(i == nkt - 1),
                        skip_group_check=True)
                ot = osmall_pool.tile([128, QR], F32, tag=f"outT{r}",
                                      name=f"ot{r}")
                nc.vector.tensor_copy(ot[:], o_ps[:])
                nc.sync.dma_start(out_d.ap()[:, QR * r:QR * (r + 1)], ot[:])

            # merged schedule: each x segment unlocks attention work
            emit_qkv(0)
            emit_vtr(0)
            emit_st(0)
            emit_qkv(1)
            emit_vtr(1)
            emit_st(1)
            emit_lsum(0)
            emit_pv(0)
            emit_qkv(2)
            emit_vtr(2)
            emit_st(2)
            emit_lsum(1)
            emit_pv(1)
            emit_qkv(3)
            emit_vtr(3)
            emit_st(3)
            emit_lsum(2)
            emit_pv(2)
            emit_lsum(3)
            emit_pv(3)

    nc.compile()
    return nc


_PROGRAM = None


def _get_program():
    global _PROGRAM
    if _PROGRAM is None:
        _PROGRAM = _build_program()
    return _PROGRAM


import ml_dtypes

BF16_NP = ml_dtypes.bfloat16


def _host_inputs(x, Wq, Wk, Wv):
    x = np.asarray(x, dtype=np.float32)
    Wq = np.asarray(Wq, dtype=np.float32)
    Wk = np.asarray(Wk, dtype=np.float32)
    Wv = np.asarray(Wv, dtype=np.float32)

    p = np.arange(128)[:, None]
    f = np.arange(128)[None, :]
    mask = (f >= p).astype(BF16_NP)
    ones = np.ones((128, 1), dtype=BF16_NP)
    wstack = np.stack([Wq, Wk, Wv])  # [3, C, H]
    wstack = wstack.reshape(3, NCC, 128, H).transpose(2, 0, 1, 3)
    wstack = np.ascontiguousarray(wstack.reshape(128, 3 * NCC * H)
                                  .astype(BF16_NP))

    in_maps = []
    for b in range(NCORES):
        xb = x[b].T.astype(BF16_NP)                       # [C, T]
        xb = xb.reshape(NCC, 128, NQR, QR).transpose(2, 1, 0, 3)
        in_maps.append({
            "x": np.ascontiguousarray(xb),
            "w": wstack, "mask": mask, "ones": ones,
        })
    return in_maps


def run(x, Wq, Wk, Wv, trace=False, **kwargs):
    nc = _get_program()
    in_maps = _host_inputs(x, Wq, Wk, Wv)
    res = run_bass_kernel_spmd(nc, in_maps, core_ids=list(range(NCORES)),
                               trace=trace, **kwargs)
    outs = []
    for b in range(NCORES):
        oT = res.results[b]["out"].astype(np.float32)     # [H, T]
        l = res.results[b]["l"].astype(np.float32)        # [1, T]
        outs.append((oT / l).T)
    return np.stack(outs, axis=0).astype(np.float32), res


def kernel(x, Wq, Wk, Wv):
    out, _ = run(x, Wq, Wk, Wv)
    return out



# revision 4
# speedup vs baseline: 1.0609x; 1.0609x over previous
"""Single-head causal attention (B=8, T=2048, C=1024, H=128) on 8 trn2 cores.

Data-parallel over batch: core b computes attention for batch element b.

v4 changes vs v3 (trace-driven):
  - DMA: balanced dual-ring posts with cc-split segment halves so each x
    segment lands just-in-time; mask rides the scalar ring; ones is a
    memset (no gpsimd SWDGE at all -> no 8us GpSimd DRAIN, Pool engine
    free for compute)
  - warmup shrunk to ~9 N=512 matmuls sized to first-data landing; PE
    then streams gap-free so the HAM clock-gate reaches 2.4 GHz at
    ~3.4us and never re-throttles (v3 lost ~8us to cold-clock QKV)
  - score strips paired into [128,2,512] PSUM tiles -> batched exp
    (1 ACT instr per full pair instead of 2)
  - diagonal E tiles are dedicated + pre-zeroed once (dead regions stay
    zero across ranges; no mid-stream memsets)
  - l ones-matmul emitted AFTER pv(r) on PE (kills the pre-l3 PE gap);
    binary tree adds on DVE at pair granularity; masks on Pool
  - engine split: ACT = exps + v copies; DVE = q/k/outT/l copies + tree;
    Pool = masks + pre-zeros

Per-core device algorithm (bf16 matmuls, f32 PSUM accum):
  1. qT/kT/vT segments [128, 512] = W.T @ xT   (8 cc chunks each)
  2. v_nat[kt] [128,128] via XBAR dma transpose of vT segment
  3. per q-range r (512 wide): strip pairs -> exp pairs -> tree -> l;
     PV accumulates strips into o_ps; outT copy + DMA per range
  4. host does (outT/l).T
"""

import numpy as np

import concourse.bacc as bacc
import concourse.mybir as mybir
import concourse.tile as tile
from concourse.bass_utils import run_bass_kernel_spmd

B, T, C, H = 8, 2048, 1024, 128
NCORES = 8
QR = 512          # q-range width (one PSUM bank)
NQR = T // QR     # 4 q-ranges
NKT = T // 128    # 16 k-strips
NCC = C // 128    # 8 contraction chunks
SCALE = 1.0 / np.sqrt(C)
NWARM = 9         # warmup matmuls (N=512, cold ~430ns each): cover DMA landing

F32 = mybir.dt.float32
BF16 = mybir.dt.bfloat16
EXP = mybir.ActivationFunctionType.Exp


def _build_program():
    nc = bacc.Bacc("TRN2", target_bir_lowering=False, debug=False,
                   num_devices=NCORES)

    # x prepped as [s][128 p][cc][512]
    x_d = nc.dram_tensor("x", [NQR, 128, NCC, QR], BF16, kind="ExternalInput")
    w_d = nc.dram_tensor("w", [128, 3 * NCC * H], BF16, kind="ExternalInput")
    mask_d = nc.dram_tensor("mask", [128, 128], BF16, kind="ExternalInput")
    out_d = nc.dram_tensor("out", [H, T], F32, kind="ExternalOutput")
    l_d = nc.dram_tensor("l", [1, T], F32, kind="ExternalOutput")

    with tile.TileContext(nc) as tc:
        with (
            tc.tile_pool(name="consts", bufs=1) as consts,
            tc.tile_pool(name="xt", bufs=NQR) as xt_pool,
            tc.tile_pool(name="qkvT", bufs=1) as qkvT_pool,
            tc.tile_pool(name="vnat", bufs=NQR) as vnat_pool,
            tc.tile_pool(name="e", bufs=8) as e_pool,
            tc.tile_pool(name="ediag", bufs=2) as ed_pool,
            tc.tile_pool(name="ep", bufs=8) as ep_pool,
            tc.tile_pool(name="lh", bufs=2) as lh_pool,
            tc.tile_pool(name="osmall", bufs=1) as osmall_pool,
            tc.tile_pool(name="mm", bufs=2, space="PSUM") as mm_pool,
            tc.tile_pool(name="st", bufs=2, space="PSUM") as st_pool,
            tc.tile_pool(name="oacc", bufs=1, space="PSUM") as oacc_pool,
            tc.tile_pool(name="lacc", bufs=1, space="PSUM") as lacc_pool,
        ):
            # ---- DMA posts FIRST so desc-gen starts at main() -------------
            w_sb = consts.tile([128, 3, NCC, H], BF16, tag="w")
            w_ap = w_d.ap().rearrange("p (w cc h) -> p w cc h", w=3, cc=NCC)
            xt = [xt_pool.tile([128, NCC, QR], BF16, tag="xt", name=f"xt{s}")
                  for s in range(NQR)]
            mask_sb = consts.tile([128, 128], BF16, tag="mask")

            # sync ring: Wq + a-halves; scalar ring: mask, Wkv + b-halves
            nc.sync.dma_start(w_sb[:, 0:1], w_ap[:, 0:1])
            nc.scalar.dma_start(mask_sb[:], mask_d.ap())
            nc.scalar.dma_start(w_sb[:, 1:3], w_ap[:, 1:3])
            for s in range(NQR):
                nc.sync.dma_start(xt[s][:, 0:4, :], x_d.ap()[s, :, 0:4, :])
                nc.scalar.dma_start(xt[s][:, 4:8, :], x_d.ap()[s, :, 4:8, :])

            # ---- local consts --------------------------------------------
            dummyw = consts.tile([128, 128], BF16, tag="dummyw")
            dummyx = consts.tile([128, QR], BF16, tag="dummyx")
            ones_sb = consts.tile([128, 1], BF16, tag="ones")
            nc.vector.memset(dummyw[:], 1.0)
            nc.vector.memset(dummyx[:], 0.0)
            nc.vector.memset(ones_sb[:], 1.0)

            # diag E tiles: dedicated, pre-zeroed once; dead regions stay 0
            # dA half1 dead [0:128]; dB half0 dead [0:256], half1 dead [0:384]
            dA = [ed_pool.tile([128, 2, QR], BF16, tag="dA", name=f"dA{i}")
                  for i in range(2)]
            dB = [ed_pool.tile([128, 2, QR], BF16, tag="dB", name=f"dB{i}")
                  for i in range(2)]
            for t_ in dA + dB:
                nc.gpsimd.memset(t_[:], 0.0)

            # ---- PE warmup (into the o-acc bank) -------------------------
            warm_ps = oacc_pool.tile([128, QR], F32, tag="o")
            for _ in range(NWARM):
                nc.tensor.matmul(warm_ps[:], dummyw[:], dummyx[:],
                                 start=True, stop=True)
            # prime the exp table early (off critical path)
            nc.scalar.activation(dummyw[:, 0:1], dummyx[:, 0:1], EXP)

            # ---- qkv + v transpose ---------------------------------------
            qTs = [qkvT_pool.tile([128, QR], BF16, tag=f"qT{s}",
                                  name=f"qT{s}") for s in range(NQR)]
            kTs = [qkvT_pool.tile([128, QR], BF16, tag=f"kT{s}",
                                  name=f"kT{s}") for s in range(NQR)]
            vTs = [qkvT_pool.tile([128, QR], BF16, tag=f"vT{s}",
                                  name=f"vT{s}") for s in range(NQR)]
            vnat = [None] * NQR

            def kslice(kt):
                return kTs[kt // 4][:, 128 * (kt % 4):128 * (kt % 4 + 1)]

            def emit_qkv(s):
                for wi, dst in ((0, qTs[s]), (1, kTs[s]), (2, vTs[s])):
                    ps = mm_pool.tile([128, QR], F32, tag="mm")
                    for cc in range(NCC):
                        nc.tensor.matmul(
                            ps[:],
                            w_sb[:, wi, cc, :],
                            xt[s][:, cc, :],
                            start=(cc == 0), stop=(cc == NCC - 1))
                    if wi == 2:
                        nc.scalar.copy(dst[:], ps[:])       # ACT (idle now)
                    else:
                        nc.vector.tensor_copy(dst[:], ps[:])  # DVE

            def emit_vtr(s):
                vt = vnat_pool.tile([128, 4, 128], BF16, tag="vnat",
                                    name=f"vnat{s}")
                nc.sync.dma_start_transpose(vt[:], vTs[s][:])
                vnat[s] = vt

            def vslice(kt):
                return vnat[kt // 4][:, kt % 4, :]

            # ---- attention ------------------------------------------------
            all_e = {}   # r -> list of (tile, half, off) per kt

            def emit_st(r):
                nkt = 4 * r + 4
                es = [None] * nkt
                pair_tiles = []
                # full strip pairs first (simple exps flow early)
                for p in range(2 * r):
                    st = st_pool.tile([128, 2, QR], F32, tag="st")
                    e = e_pool.tile([128, 2, QR], BF16, tag="e",
                                    name=f"e{r}_{p}")
                    for half in range(2):
                        kt = 2 * p + half
                        nc.tensor.matmul(
                            st[:, half, :], kslice(kt), qTs[r][:],
                            start=True, stop=True, skip_group_check=True)
                        es[kt] = (e, half, 0)
                    nc.scalar.activation(e[:], st[:], EXP, scale=float(SCALE))
                    pair_tiles.append(e)
                # diagonal pairs: dA = (j0, j1), dB = (j2, j3)
                for di, dt_ in ((0, dA[r % 2]), (1, dB[r % 2])):
                    st = st_pool.tile([128, 2, QR], F32, tag="st")
                    for half in range(2):
                        j = 2 * di + half
                        kt = 4 * r + j
                        off = 128 * j
                        nc.tensor.matmul(
                            st[:, half, off:QR], kslice(kt), qTs[r][:, off:QR],
                            start=True, stop=True, skip_group_check=True)
                        nc.scalar.activation(
                            dt_[:, half, off:QR], st[:, half, off:QR],
                            EXP, scale=float(SCALE))
                        # triangular mask on Pool
                        nc.gpsimd.tensor_mul(
                            dt_[:, half, off:off + 128],
                            dt_[:, half, off:off + 128],
                            mask_sb[:])
                        es[kt] = (dt_, half, off)
                    pair_tiles.append(dt_)
                all_e[r] = es

                # l tree: pairwise adds on DVE down to one [128,2,QR] root
                lvl = pair_tiles
                while len(lvl) > 1:
                    nxt = []
                    for i in range(0, len(lvl) - 1, 2):
                        d = ep_pool.tile([128, 2, QR], BF16, tag="ep")
                        nc.vector.tensor_add(d[:], lvl[i][:], lvl[i + 1][:])
                        nxt.append(d)
                    if len(lvl) % 2:
                        nxt.append(lvl[-1])
                    lvl = nxt
                root = lvl[0]
                lh = lh_pool.tile([128, QR], BF16, tag="lh")
                nc.vector.tensor_add(lh[:], root[:, 0, :], root[:, 1, :])
                all_e[r] = (es, lh)

            l_sb = osmall_pool.tile([1, T], F32, tag="l_sb")

            def emit_pv(r):
                nkt = 4 * r + 4
                es, lh = all_e.pop(r)
                o_ps = oacc_pool.tile([128, QR], F32, tag="o")
                for kt in range(nkt):
                    e, half, off = es[kt]
                    nc.tensor.matmul(
                        o_ps[:, off:QR],
                        vslice(kt),
                        e[:, half, off:QR],
                        start=(kt == 0), stop=(kt == nkt - 1),
                        skip_group_check=True)
                ot = osmall_pool.tile([128, QR], F32, tag=f"outT{r}",
                                      name=f"ot{r}")
                nc.vector.tensor_copy(ot[:], o_ps[:])
                nc.sync.dma_start(out_d.ap()[:, QR * r:QR * (r + 1)], ot[:])
                # l AFTER pv on PE so PV never waits on the tree
                l_ps = lacc_pool.tile([1, QR], F32, tag="l")
                nc.tensor.matmul(l_ps[:], ones_sb[:], lh[:],
                                 start=True, stop=True)
                nc.vector.tensor_copy(l_sb[:, QR * r:QR * (r + 1)], l_ps[:])

            # ---- schedule: QKV prioritized, attention streams behind -----
            emit_qkv(0)
            emit_vtr(0)
            emit_qkv(1)
            emit_vtr(1)
            emit_qkv(2)
            emit_vtr(2)
            emit_st(0)
            emit_pv(0)
            emit_qkv(3)
            emit_vtr(3)
            emit_st(1)
            emit_pv(1)
            emit_st(2)
            emit_pv(2)
            emit_st(3)
            emit_pv(3)
            nc.sync.dma_start(l_d.ap()[:], l_sb[:])

    nc.compile()
    return nc


_PROGRAM = None


def _get_program():
    global _PROGRAM
    if _PROGRAM is None:
        _PROGRAM = _build_program()
    return _PROGRAM


import ml_dtypes

BF16_NP = ml_dtypes.bfloat16


def _host_inputs(x, Wq, Wk, Wv):
    x = np.asarray(x, dtype=np.float32)
    Wq = np.asarray(Wq, dtype=np.float32)
    Wk = np.asarray(Wk, dtype=np.float32)
    Wv = np.asarray(Wv, dtype=np.float32)

    p = np.arange(128)[:, None]
    f = np.arange(128)[None, :]
    mask = (f >= p).astype(BF16_NP)
    wstack = np.stack([Wq, Wk, Wv])  # [3, C, H]
    wstack = wstack.reshape(3, NCC, 128, H).transpose(2, 0, 1, 3)
    wstack = np.ascontiguousarray(wstack.reshape(128, 3 * NCC * H)
                                  .astype(BF16_NP))

    in_maps = []
    for b in range(NCORES):
        xb = x[b].T.astype(BF16_NP)                       # [C, T]
        xb = xb.reshape(NCC, 128, NQR, QR).transpose(2, 1, 0, 3)
        in_maps.append({
            "x": np.ascontiguousarray(xb),
            "w": wstack, "mask": mask,
        })
    return in_maps


def run(x, Wq, Wk, Wv, trace=False, **kwargs):
    nc = _get_program()
    in_maps = _host_inputs(x, Wq, Wk, Wv)
    res = run_bass_kernel_spmd(nc, in_maps, core_ids=list(range(NCORES)),
                               trace=trace, **kwargs)
    outs = []
    for b in range(NCORES):
        oT = res.results[b]["out"].astype(np.float32)     # [H, T]
        l = res.results[b]["l"].astype(np.float32)        # [1, T]
        outs.append((oT / l).T)
    return np.stack(outs, axis=0).astype(np.float32), res


def kernel(x, Wq, Wk, Wv):
    out, _ = run(x, Wq, Wk, Wv)
    return out


# revision 5
# speedup vs baseline: 1.2137x; 1.1440x over previous
"""Single-head causal attention (B=8, T=2048, C=1024, H=128) on 8 trn2 cores.

Data-parallel over batch: core b computes attention for batch element b.

v5 changes vs v4 (trace-driven):
  - x loaded in segment order 0,3,1,2; x0 split into four cc-pair chunks
    across both rings so QKV(0) streams gap-free from ~13us; warmup sized
    to that (NWARM=13) so the HAM clock-gate hits 2.4 GHz during warmup
    and never re-throttles
  - st(3) split into parts emitted right after qkv3/qkv1/qkv2 so the
    9us exp burst of range 3 spreads across the whole middle of the
    kernel instead of gating pv3 at the end
  - l partial sums via a serial bf16 accumulator chain (adds emitted
    right after each exp pair -> no DVE FIFO head-blocking)
  - all PSUM->SBUF copies on DVE; ACT does exps only
  - dedicated pre-zeroed diag E tiles per range (no cross-range WAR)

Per-core device algorithm (bf16 matmuls, f32 PSUM accum):
  1. qT/kT/vT segments [128, 512] = W.T @ xT   (8 cc chunks each)
  2. v_nat[kt] [128,128] via XBAR dma transpose of vT segment
  3. per q-range r (512 wide): strip pairs -> batched exp -> acc chain;
     PV accumulates strips into o_ps; l = ones.T @ acc after pv
  4. host does (outT/l).T
"""

import numpy as np

import concourse.bacc as bacc
import concourse.mybir as mybir
import concourse.tile as tile
from concourse.bass_utils import run_bass_kernel_spmd

B, T, C, H = 8, 2048, 1024, 128
NCORES = 8
QR = 512          # q-range width (one PSUM bank)
NQR = T // QR     # 4 q-ranges
NKT = T // 128    # 16 k-strips
NCC = C // 128    # 8 contraction chunks
SCALE = 1.0 / np.sqrt(C)
NWARM = 13        # warmup matmuls (N=512): cover DMA landing of x0

F32 = mybir.dt.float32
BF16 = mybir.dt.bfloat16
EXP = mybir.ActivationFunctionType.Exp


def _build_program():
    nc = bacc.Bacc("TRN2", target_bir_lowering=False, debug=False,
                   num_devices=NCORES)

    # x prepped as [s][128 p][cc][512]
    x_d = nc.dram_tensor("x", [NQR, 128, NCC, QR], BF16, kind="ExternalInput")
    w_d = nc.dram_tensor("w", [128, 3 * NCC * H], BF16, kind="ExternalInput")
    mask_d = nc.dram_tensor("mask", [128, 128], BF16, kind="ExternalInput")
    out_d = nc.dram_tensor("out", [H, T], F32, kind="ExternalOutput")
    l_d = nc.dram_tensor("l", [1, T], F32, kind="ExternalOutput")

    with tile.TileContext(nc) as tc:
        with (
            tc.tile_pool(name="consts", bufs=1) as consts,
            tc.tile_pool(name="xt", bufs=NQR) as xt_pool,
            tc.tile_pool(name="qkvT", bufs=1) as qkvT_pool,
            tc.tile_pool(name="vnat", bufs=NQR) as vnat_pool,
            tc.tile_pool(name="e", bufs=12) as e_pool,
            tc.tile_pool(name="ediag", bufs=1) as ed_pool,
            tc.tile_pool(name="ep", bufs=8) as ep_pool,
            tc.tile_pool(name="lh", bufs=2) as lh_pool,
            tc.tile_pool(name="osmall", bufs=1) as osmall_pool,
            tc.tile_pool(name="mm", bufs=2, space="PSUM") as mm_pool,
            tc.tile_pool(name="st", bufs=2, space="PSUM") as st_pool,
            tc.tile_pool(name="oacc", bufs=1, space="PSUM") as oacc_pool,
            tc.tile_pool(name="lacc", bufs=1, space="PSUM") as lacc_pool,
        ):
            # ---- DMA posts FIRST so desc-gen starts at main() -------------
            w_sb = consts.tile([128, 3, NCC, H], BF16, tag="w")
            w_ap = w_d.ap().rearrange("p (w cc h) -> p w cc h", w=3, cc=NCC)
            xt = [xt_pool.tile([128, NCC, QR], BF16, tag="xt", name=f"xt{s}")
                  for s in range(NQR)]
            mask_sb = consts.tile([128, 128], BF16, tag="mask")

            # sync ring: Wq, x0 cc01/cc45, x3a, x1a, x2a
            # scalar ring: mask, Wkv, x0 cc23/cc67, x3b, x1b, x2b
            nc.scalar.dma_start(mask_sb[:], mask_d.ap())
            nc.sync.dma_start(w_sb[:, 0:1], w_ap[:, 0:1])
            nc.scalar.dma_start(w_sb[:, 1:3], w_ap[:, 1:3])
            nc.sync.dma_start(xt[0][:, 0:2, :], x_d.ap()[0, :, 0:2, :])
            nc.scalar.dma_start(xt[0][:, 2:4, :], x_d.ap()[0, :, 2:4, :])
            nc.sync.dma_start(xt[0][:, 4:6, :], x_d.ap()[0, :, 4:6, :])
            nc.scalar.dma_start(xt[0][:, 6:8, :], x_d.ap()[0, :, 6:8, :])
            for s in (3, 1, 2):
                nc.sync.dma_start(xt[s][:, 0:4, :], x_d.ap()[s, :, 0:4, :])
                nc.scalar.dma_start(xt[s][:, 4:8, :], x_d.ap()[s, :, 4:8, :])

            # ---- local consts --------------------------------------------
            dummyw = consts.tile([128, 128], BF16, tag="dummyw")
            dummyx = consts.tile([128, QR], BF16, tag="dummyx")
            ones_sb = consts.tile([128, 1], BF16, tag="ones")
            nc.vector.memset(dummyw[:], 1.0)
            nc.vector.memset(dummyx[:], 0.0)
            nc.vector.memset(ones_sb[:], 1.0)

            # diag E tiles: dedicated per range, pre-zeroed once
            # dA half1 dead [0:128]; dB half0 dead [0:256], half1 dead [0:384]
            dA = [ed_pool.tile([128, 2, QR], BF16, tag=f"dA{r}",
                               name=f"dA{r}") for r in range(NQR)]
            dB = [ed_pool.tile([128, 2, QR], BF16, tag=f"dB{r}",
                               name=f"dB{r}") for r in range(NQR)]
            for t_ in dA + dB:
                nc.gpsimd.memset(t_[:], 0.0)

            # ---- PE warmup (into the o-acc bank) -------------------------
            warm_ps = oacc_pool.tile([128, QR], F32, tag="o")
            for _ in range(NWARM):
                nc.tensor.matmul(warm_ps[:], dummyw[:], dummyx[:],
                                 start=True, stop=True)
            # prime the exp table early (off critical path)
            nc.scalar.activation(dummyw[:, 0:1], dummyx[:, 0:1], EXP)

            # ---- qkv + v transpose ---------------------------------------
            qTs = [qkvT_pool.tile([128, QR], BF16, tag=f"qT{s}",
                                  name=f"qT{s}") for s in range(NQR)]
            kTs = [qkvT_pool.tile([128, QR], BF16, tag=f"kT{s}",
                                  name=f"kT{s}") for s in range(NQR)]
            vTs = [qkvT_pool.tile([128, QR], BF16, tag=f"vT{s}",
                                  name=f"vT{s}") for s in range(NQR)]
            vnat = [None] * NQR

            def kslice(kt):
                return kTs[kt // 4][:, 128 * (kt % 4):128 * (kt % 4 + 1)]

            def emit_qkv(s):
                for wi, dst in ((0, qTs[s]), (1, kTs[s]), (2, vTs[s])):
                    ps = mm_pool.tile([128, QR], F32, tag="mm")
                    for cc in range(NCC):
                        nc.tensor.matmul(
                            ps[:],
                            w_sb[:, wi, cc, :],
                            xt[s][:, cc, :],
                            start=(cc == 0), stop=(cc == NCC - 1))
                    nc.vector.tensor_copy(dst[:], ps[:])

            def emit_vtr(s):
                vt = vnat_pool.tile([128, 4, 128], BF16, tag="vnat",
                                    name=f"vnat{s}")
                nc.sync.dma_start_transpose(vt[:], vTs[s][:])
                vnat[s] = vt

            def vslice(kt):
                return vnat[kt // 4][:, kt % 4, :]

            # ---- attention ------------------------------------------------
            # per range: es[kt] = (tile, half, off); lacc = running bf16 sum
            es_all = {r: [None] * (4 * r + 4) for r in range(NQR)}
            lacc_sb = {}   # r -> current accumulator tile (or pending pair)

            def _lacc_add(r, pair_tile):
                prev = lacc_sb.get(r)
                if prev is None:
                    lacc_sb[r] = pair_tile
                else:
                    d = ep_pool.tile([128, 2, QR], BF16, tag="ep")
                    nc.vector.tensor_add(d[:], prev[:], pair_tile[:])
                    lacc_sb[r] = d

            def emit_st_full(r, pairs):
                """Emit full strip-pairs `pairs` (list of pair indices)."""
                for p in pairs:
                    st = st_pool.tile([128, 2, QR], F32, tag="st")
                    e = e_pool.tile([128, 2, QR], BF16, tag="e",
                                    name=f"e{r}_{p}")
                    for half in range(2):
                        kt = 2 * p + half
                        nc.tensor.matmul(
                            st[:, half, :], kslice(kt), qTs[r][:],
                            start=True, stop=True, skip_group_check=True)
                        es_all[r][kt] = (e, half, 0)
                    nc.scalar.activation(e[:], st[:], EXP, scale=float(SCALE))
                    _lacc_add(r, e)

            def emit_st_diag(r):
                for di, dt_ in ((0, dA[r]), (1, dB[r])):
                    st = st_pool.tile([128, 2, QR], F32, tag="st")
                    for half in range(2):
                        j = 2 * di + half
                        kt = 4 * r + j
                        off = 128 * j
                        nc.tensor.matmul(
                            st[:, half, off:QR], kslice(kt), qTs[r][:, off:QR],
                            start=True, stop=True, skip_group_check=True)
                        nc.scalar.activation(
                            dt_[:, half, off:QR], st[:, half, off:QR],
                            EXP, scale=float(SCALE))
                        nc.gpsimd.tensor_mul(
                            dt_[:, half, off:off + 128],
                            dt_[:, half, off:off + 128],
                            mask_sb[:])
                        es_all[r][kt] = (dt_, half, off)
                    _lacc_add(r, dt_)

            l_sb = osmall_pool.tile([1, T], F32, tag="l_sb")

            def emit_pv(r):
                nkt = 4 * r + 4
                es = es_all[r]
                o_ps = oacc_pool.tile([128, QR], F32, tag="o")
                for kt in range(nkt):
                    e, half, off = es[kt]
                    nc.tensor.matmul(
                        o_ps[:, off:QR],
                        vslice(kt),
                        e[:, half, off:QR],
                        start=(kt == 0), stop=(kt == nkt - 1),
                        skip_group_check=True)
                # l right after pv on PE
                root = lacc_sb[r]
                lh = lh_pool.tile([128, QR], BF16, tag="lh")
                nc.vector.tensor_add(lh[:], root[:, 0, :], root[:, 1, :])
                l_ps = lacc_pool.tile([1, QR], F32, tag="l")
                nc.tensor.matmul(l_ps[:], ones_sb[:], lh[:],
                                 start=True, stop=True)
                ot = osmall_pool.tile([128, QR], F32, tag=f"outT{r}",
                                      name=f"ot{r}")
                nc.vector.tensor_copy(ot[:], o_ps[:])
                nc.sync.dma_start(out_d.ap()[:, QR * r:QR * (r + 1)], ot[:])
                nc.vector.tensor_copy(l_sb[:, QR * r:QR * (r + 1)], l_ps[:])

            # ---- schedule -------------------------------------------------
            emit_qkv(0)
            emit_vtr(0)
            emit_st_diag(0)
            emit_qkv(3)
            emit_vtr(3)
            emit_st_full(3, [0, 1])      # kt 0-3 (needs kT seg0, qT3)
            emit_st_diag(3)              # kt 12-15 (needs kT seg3)
            emit_qkv(1)
            emit_vtr(1)
            emit_st_full(1, [0, 1])
            emit_st_diag(1)
            emit_st_full(3, [2, 3])      # kt 4-7 (needs kT seg1)
            emit_qkv(2)
            emit_vtr(2)
            emit_st_full(2, [0, 1, 2, 3])
            emit_st_diag(2)
            emit_st_full(3, [4, 5])      # kt 8-11 (needs kT seg2)
            emit_pv(0)
            emit_pv(1)
            emit_pv(2)
            emit_pv(3)
            nc.sync.dma_start(l_d.ap()[:], l_sb[:])

    nc.compile()
    return nc


_PROGRAM = None


def _get_program():
    global _PROGRAM
    if _PROGRAM is None:
        _PROGRAM = _build_program()
    return _PROGRAM


import ml_dtypes

BF16_NP = ml_dtypes.bfloat16


def _host_inputs(x, Wq, Wk, Wv):
    x = np.asarray(x, dtype=np.float32)
    Wq = np.asarray(Wq, dtype=np.float32)
    Wk = np.asarray(Wk, dtype=np.float32)
    Wv = np.asarray(Wv, dtype=np.float32)

    p = np.arange(128)[:, None]
    f = np.arange(128)[None, :]
    mask = (f >= p).astype(BF16_NP)
    wstack = np.stack([Wq, Wk, Wv])  # [3, C, H]
    wstack = wstack.reshape(3, NCC, 128, H).transpose(2, 0, 1, 3)
    wstack = np.ascontiguousarray(wstack.reshape(128, 3 * NCC * H)
                                  .astype(BF16_NP))

    in_maps = []
    for b in range(NCORES):
        xb = x[b].T.astype(BF16_NP)                       # [C, T]
        xb = xb.reshape(NCC, 128, NQR, QR).transpose(2, 1, 0, 3)
        in_maps.append({
            "x": np.ascontiguousarray(xb),
            "w": wstack, "mask": mask,
        })
    return in_maps


def run(x, Wq, Wk, Wv, trace=False, **kwargs):
    nc = _get_program()
    in_maps = _host_inputs(x, Wq, Wk, Wv)
    res = run_bass_kernel_spmd(nc, in_maps, core_ids=list(range(NCORES)),
                               trace=trace, **kwargs)
    outs = []
    for b in range(NCORES):
        oT = res.results[b]["out"].astype(np.float32)     # [H, T]
        l = res.results[b]["l"].astype(np.float32)        # [1, T]
        outs.append((oT / l).T)
    return np.stack(outs, axis=0).astype(np.float32), res


def kernel(x, Wq, Wk, Wv):
    out, _ = run(x, Wq, Wk, Wv)
    return out


# revision 8
# speedup vs baseline: 1.3262x; 1.0927x over previous
"""Single-head causal attention (B=8, T=2048, C=1024, H=128) on 8 trn2 cores.

Data-parallel over batch: core b computes attention for batch element b.

v6 changes vs v5 (trace-driven):
  - W stored cc-major; W+x0 posted as interleaved cc-pair chunks
    alternating rings, so QKV(0) can start ~10us and stream cc-major
    at the HBM feed rate (v5 waited until ~17us for x0)
  - QKV(0) emitted cc-major with q/k/v accumulating in 3 concurrent
    PSUM tiles; NWARM=6
  - o_ps double-buffered (pv(r+1) no longer waits outT(r) copy)
  - output DMAs split across both rings; l copy before ot3 copy so the
    two tail DMA desc-gens overlap
  - rest as v5: st(3) split across the middle, serial l-accumulator,
    batched pair exps, dedicated pre-zeroed diag tiles
"""

import numpy as np

import concourse.bacc as bacc
import concourse.mybir as mybir
import concourse.tile as tile
from concourse.bass_utils import run_bass_kernel_spmd

B, T, C, H = 8, 2048, 1024, 128
NCORES = 8
QR = 512          # q-range width (one PSUM bank)
NQR = T // QR     # 4 q-ranges
NKT = T // 128    # 16 k-strips
NCC = C // 128    # 8 contraction chunks
SCALE = 1.0 / np.sqrt(C)
NWARM = 6         # warmup matmuls (N=512): cover until first cc chunks land

F32 = mybir.dt.float32
BF16 = mybir.dt.bfloat16
EXP = mybir.ActivationFunctionType.Exp


def _build_program():
    nc = bacc.Bacc("TRN2", target_bir_lowering=False, debug=False,
                   num_devices=NCORES)

    # x prepped as [s][128 p][cc][512]; W cc-major [128, cc, 3, H]
    x_d = nc.dram_tensor("x", [NQR, 128, NCC, QR], BF16, kind="ExternalInput")
    w_d = nc.dram_tensor("w", [128, 3 * NCC * H], BF16, kind="ExternalInput")
    mask_d = nc.dram_tensor("mask", [128, 128], BF16, kind="ExternalInput")
    out_d = nc.dram_tensor("out", [H, T], F32, kind="ExternalOutput")
    l_d = nc.dram_tensor("l", [1, T], F32, kind="ExternalOutput")

    with tile.TileContext(nc) as tc:
        with (
            tc.tile_pool(name="consts", bufs=1) as consts,
            tc.tile_pool(name="xt", bufs=NQR) as xt_pool,
            tc.tile_pool(name="qkvT", bufs=1) as qkvT_pool,
            tc.tile_pool(name="vnat", bufs=NQR) as vnat_pool,
            tc.tile_pool(name="e", bufs=12) as e_pool,
            tc.tile_pool(name="ediag", bufs=1) as ed_pool,
            tc.tile_pool(name="ep", bufs=8) as ep_pool,
            tc.tile_pool(name="lh", bufs=2) as lh_pool,
            tc.tile_pool(name="osmall", bufs=1) as osmall_pool,
            tc.tile_pool(name="mm", bufs=2, space="PSUM") as mm_pool,
            tc.tile_pool(name="st", bufs=2, space="PSUM") as st_pool,
            tc.tile_pool(name="oacc", bufs=2, space="PSUM") as oacc_pool,
        ):
            # ---- DMA posts FIRST so desc-gen starts at main() -------------
            w_sb = consts.tile([128, NCC, 3, H], BF16, tag="w")
            w_ap = w_d.ap().rearrange("p (cc w h) -> p cc w h", cc=NCC, w=3)
            xt = [xt_pool.tile([128, NCC, QR], BF16, tag="xt", name=f"xt{s}")
                  for s in range(NQR)]
            mask_sb = consts.tile([128, 128], BF16, tag="mask")

            # W+x0 cc-pairs interleaved, alternating rings, consumption order
            nc.scalar.dma_start(mask_sb[:], mask_d.ap())
            nc.sync.dma_start(w_sb[:, 0:2], w_ap[:, 0:2])
            nc.scalar.dma_start(w_sb[:, 2:4], w_ap[:, 2:4])
            nc.sync.dma_start(xt[0][:, 0:2, :], x_d.ap()[0, :, 0:2, :])
            nc.scalar.dma_start(xt[0][:, 2:4, :], x_d.ap()[0, :, 2:4, :])
            nc.sync.dma_start(w_sb[:, 4:6], w_ap[:, 4:6])
            nc.scalar.dma_start(w_sb[:, 6:8], w_ap[:, 6:8])
            nc.sync.dma_start(xt[0][:, 4:6, :], x_d.ap()[0, :, 4:6, :])
            nc.scalar.dma_start(xt[0][:, 6:8, :], x_d.ap()[0, :, 6:8, :])
            for s in (3, 1, 2):
                nc.sync.dma_start(xt[s][:, 0:4, :], x_d.ap()[s, :, 0:4, :])
                nc.scalar.dma_start(xt[s][:, 4:8, :], x_d.ap()[s, :, 4:8, :])

            # ---- local consts --------------------------------------------
            dummyw = consts.tile([128, 128], BF16, tag="dummyw")
            dummyx = consts.tile([128, QR], BF16, tag="dummyx")
            ones_sb = consts.tile([128, 1], BF16, tag="ones")
            nc.vector.memset(dummyw[:], 1.0)
            nc.vector.memset(dummyx[:], 0.0)
            nc.vector.memset(ones_sb[:], 1.0)

            # diag E tiles: dedicated per range, pre-zeroed once
            dA = [ed_pool.tile([128, 2, QR], BF16, tag=f"dA{r}",
                               name=f"dA{r}") for r in range(NQR)]
            dB = [ed_pool.tile([128, 2, QR], BF16, tag=f"dB{r}",
                               name=f"dB{r}") for r in range(NQR)]
            for t_ in dA + dB:
                nc.gpsimd.memset(t_[:], 0.0)

            # ---- PE warmup (into the o-acc bank) -------------------------
            warm_ps = oacc_pool.tile([128, QR], F32, tag="o")
            for _ in range(NWARM):
                nc.tensor.matmul(warm_ps[:], dummyw[:], dummyx[:],
                                 start=True, stop=True)
            nc.scalar.activation(dummyw[:, 0:1], dummyx[:, 0:1], EXP)

            # ---- qkv + v transpose ---------------------------------------
            qTs = [qkvT_pool.tile([128, QR], BF16, tag=f"qT{s}",
                                  name=f"qT{s}") for s in range(NQR)]
            kTs = [qkvT_pool.tile([128, QR], BF16, tag=f"kT{s}",
                                  name=f"kT{s}") for s in range(NQR)]
            vTs = [qkvT_pool.tile([128, QR], BF16, tag=f"vT{s}",
                                  name=f"vT{s}") for s in range(NQR)]
            vnat = [None] * NQR

            def kslice(kt):
                return kTs[kt // 4][:, 128 * (kt % 4):128 * (kt % 4 + 1)]

            def emit_qkv0_ccmajor():
                """QKV(0) cc-major: consume x0/W chunks as they land."""
                psq = mm_pool.tile([128, QR], F32, tag="mm")
                psk = mm_pool.tile([128, QR], F32, tag="mm")
                psv = oacc_pool.tile([128, QR], F32, tag="o")
                for cc in range(NCC):
                    for wi, ps in ((0, psq), (1, psk), (2, psv)):
                        nc.tensor.matmul(
                            ps[:], w_sb[:, cc, wi, :], xt[0][:, cc, :],
                            start=(cc == 0), stop=(cc == NCC - 1))
                nc.vector.tensor_copy(qTs[0][:], psq[:])
                nc.vector.tensor_copy(kTs[0][:], psk[:])
                nc.vector.tensor_copy(vTs[0][:], psv[:])

            def emit_qkv(s):
                for wi, dst in ((0, qTs[s]), (1, kTs[s]), (2, vTs[s])):
                    ps = mm_pool.tile([128, QR], F32, tag="mm")
                    for cc in range(NCC):
                        nc.tensor.matmul(
                            ps[:],
                            w_sb[:, cc, wi, :],
                            xt[s][:, cc, :],
                            start=(cc == 0), stop=(cc == NCC - 1))
                    nc.vector.tensor_copy(dst[:], ps[:])

            def emit_vtr(s):
                vt = vnat_pool.tile([128, 4, 128], BF16, tag="vnat",
                                    name=f"vnat{s}")
                nc.sync.dma_start_transpose(vt[:], vTs[s][:])
                vnat[s] = vt

            def vslice(kt):
                return vnat[kt // 4][:, kt % 4, :]

            # ---- attention ------------------------------------------------
            es_all = {r: [None] * (4 * r + 4) for r in range(NQR)}
            lacc_sb = {}

            def _lacc_add(r, pair_tile):
                prev = lacc_sb.get(r)
                if prev is None:
                    lacc_sb[r] = pair_tile
                else:
                    d = ep_pool.tile([128, 2, QR], BF16, tag="ep")
                    nc.vector.tensor_add(d[:], prev[:], pair_tile[:])
                    lacc_sb[r] = d

            def emit_st_full(r, pairs):
                for p in pairs:
                    st = st_pool.tile([128, 2, QR], F32, tag="st")
                    e = e_pool.tile([128, 2, QR], BF16, tag="e",
                                    name=f"e{r}_{p}")
                    for half in range(2):
                        kt = 2 * p + half
                        nc.tensor.matmul(
                            st[:, half, :], kslice(kt), qTs[r][:],
                            start=True, stop=True, skip_group_check=True)
                        es_all[r][kt] = (e, half, 0)
                    nc.scalar.activation(e[:], st[:], EXP, scale=float(SCALE))
                    _lacc_add(r, e)

            def emit_st_diag(r):
                for di, dt_ in ((0, dA[r]), (1, dB[r])):
                    st = st_pool.tile([128, 2, QR], F32, tag="st")
                    for half in range(2):
                        j = 2 * di + half
                        kt = 4 * r + j
                        off = 128 * j
                        nc.tensor.matmul(
                            st[:, half, off:QR], kslice(kt), qTs[r][:, off:QR],
                            start=True, stop=True, skip_group_check=True)
                        nc.scalar.activation(
                            dt_[:, half, off:QR], st[:, half, off:QR],
                            EXP, scale=float(SCALE))
                        nc.gpsimd.tensor_mul(
                            dt_[:, half, off:off + 128],
                            dt_[:, half, off:off + 128],
                            mask_sb[:])
                        es_all[r][kt] = (dt_, half, off)
                    _lacc_add(r, dt_)

            l_sb = osmall_pool.tile([1, T], F32, tag="l_sb")

            def emit_pv(r):
                nkt = 4 * r + 4
                es = es_all[r]
                o_ps = oacc_pool.tile([128, QR], F32, tag="o")
                for kt in range(nkt):
                    e, half, off = es[kt]
                    nc.tensor.matmul(
                        o_ps[:, off:QR],
                        vslice(kt),
                        e[:, half, off:QR],
                        start=(kt == 0), stop=(kt == nkt - 1),
                        skip_group_check=True)
                root = lacc_sb[r]
                lh = lh_pool.tile([128, QR], BF16, tag="lh")
                nc.vector.tensor_add(lh[:], root[:, 0, :], root[:, 1, :])
                l_ps = mm_pool.tile([1, QR], F32, tag="mm")
                nc.tensor.matmul(l_ps[:], ones_sb[:], lh[:],
                                 start=True, stop=True)
                # l copy before ot copy so tail desc-gens overlap
                nc.vector.tensor_copy(l_sb[:, QR * r:QR * (r + 1)], l_ps[:])
                ot = osmall_pool.tile([128, QR], F32, tag=f"outT{r}",
                                      name=f"ot{r}")
                nc.vector.tensor_copy(ot[:], o_ps[:])
                ring = nc.sync if r % 2 == 0 else nc.scalar
                ring.dma_start(out_d.ap()[:, QR * r:QR * (r + 1)], ot[:])

            # ---- schedule -------------------------------------------------
            emit_qkv0_ccmajor()
            emit_vtr(0)
            emit_st_diag(0)
            emit_qkv(3)
            emit_vtr(3)
            emit_st_full(3, [0, 1])      # kt 0-3
            emit_st_diag(3)              # kt 12-15
            emit_qkv(1)
            emit_vtr(1)
            emit_st_full(1, [0, 1])
            emit_st_diag(1)
            emit_st_full(3, [2, 3])      # kt 4-7
            emit_qkv(2)
            emit_vtr(2)
            emit_st_full(2, [0, 1, 2, 3])
            emit_st_diag(2)
            emit_st_full(3, [4, 5])      # kt 8-11
            emit_pv(0)
            emit_pv(1)
            emit_pv(2)
            emit_pv(3)
            nc.sync.dma_start(l_d.ap()[:], l_sb[:])

    nc.compile()
    return nc


_PROGRAM = None


def _get_program():
    global _PROGRAM
    if _PROGRAM is None:
        _PROGRAM = _build_program()
    return _PROGRAM


import ml_dtypes

BF16_NP = ml_dtypes.bfloat16


def _host_inputs(x, Wq, Wk, Wv):
    x = np.asarray(x, dtype=np.float32)
    Wq = np.asarray(Wq, dtype=np.float32)
    Wk = np.asarray(Wk, dtype=np.float32)
    Wv = np.asarray(Wv, dtype=np.float32)

    p = np.arange(128)[:, None]
    f = np.arange(128)[None, :]
    mask = (f >= p).astype(BF16_NP)
    wstack = np.stack([Wq, Wk, Wv])  # [3, C, H]
    # cc-major: [128, cc, 3, H]
    wstack = wstack.reshape(3, NCC, 128, H).transpose(2, 1, 0, 3)
    wstack = np.ascontiguousarray(wstack.reshape(128, 3 * NCC * H)
                                  .astype(BF16_NP))

    in_maps = []
    for b in range(NCORES):
        xb = x[b].T.astype(BF16_NP)                       # [C, T]
        xb = xb.reshape(NCC, 128, NQR, QR).transpose(2, 1, 0, 3)
        in_maps.append({
            "x": np.ascontiguousarray(xb),
            "w": wstack, "mask": mask,
        })
    return in_maps


def run(x, Wq, Wk, Wv, trace=False, **kwargs):
    nc = _get_program()
    in_maps = _host_inputs(x, Wq, Wk, Wv)
    res = run_bass_kernel_spmd(nc, in_maps, core_ids=list(range(NCORES)),
                               trace=trace, **kwargs)
    outs = []
    for b in range(NCORES):
        oT = res.results[b]["out"].astype(np.float32)     # [H, T]
        l = res.results[b]["l"].astype(np.float32)        # [1, T]
        outs.append((oT / l).T)
    return np.stack(outs, axis=0).astype(np.float32), res


def kernel(x, Wq, Wk, Wv):
    out, _ = run(x, Wq, Wk, Wv)
    return out
